# revision 46
# baseline (speedup 1.0000x reference)
"""MoE LoRA delta kernel for Trainium2 (Bass/Tile), 8-core SPMD.

Computation (reference):
  flat [T,F] -> logits = flat @ Wr.T [T,E]; top-2 softmax gates
  mid = flat @ A[e].T  [T,E,R];  delta = sum_e gates[:,e] * (mid[e] @ B[e].T) * SCALE

Shapes: T=4096 (2x2048), F=O=4096, E=4, R=16, SCALE=4.0.

Strategy:
  - Token-shard T across 8 cores (512 tokens each); replicate tiny weights.
  - x split host-side into bf16 hi/lo planes (error-free to ~2^-18),
    packed half-major into 4 contiguous 2MB blocks (h0_xh, h0_xl,
    h1_xh, h1_xl) and loaded with HW DMA-transpose (X-bar), so NO PE
    transposes are needed. All transposes stay on ONE HWDGE ring
    (concurrent X-bar transposes from both rings corrupt data) and
    issue back-to-back behind a strict barrier (Tile serializes
    DMA_TRANSPOSE against any concurrent DMA bidirectionally; output
    stores carry an explicit dep edge past the last transpose).
  - Tokens run in 2 halves of 256: half-0's gates + mm2 + PSUM->SBUF
    copies overlap half-1's transpose window, which also keeps the PE
    p-state hot into phase 2. Both phase-1 chains are emitted before any
    gates/mm2 so the scheduler prioritizes the half-1 chain (critical
    path) over mm2-h0 (which has copy/DMA slack).
  - Phase 1 (per half): one 64-matmul bf16 PSUM chain computes mid[64]
    AND router logits split across rows 64:68 (x@wrh) + 68:72 (xh@wrl):
      S1 = [Ah | wrh | wrl] vs xh;  S2 = [Ah | wrh | 0] vs xl.
    Logits are exact to ~2e-5 (top-2 margins on these inputs are >=7e-5,
    so top-2 selection matches the fp32 reference deterministically).
  - Gates: the hi+lo logit sum is folded into the to-[t,e] transpose via
    a [I4;I4] matmul; top-2 softmax batched across token chunks
    (9 wide DVE/ACT ops, exp without max-shift since softmax normalizes);
    transpose back and replicate rows 4->64 with a 0/1 matmul (fp32r).
  - Phase 2: delta tiles [128t, 512o] = (gates*mid).T-chunks @ B_cat.T in
    fp32r (full PE rate at N>=512); SCALE folded into B host-side.
"""

import numpy as np
import ml_dtypes

import concourse.bass as bass
import concourse.mybir as mybir
import concourse.tile as tile
from concourse import bacc, masks
from concourse.bass_utils import run_bass_kernel_spmd
from concourse.tile_rust import add_dep_helper

F32 = mybir.dt.float32
F32R = mybir.dt.float32r
BF16 = mybir.dt.bfloat16
NP_BF16 = ml_dtypes.bfloat16

N_CORES = 8
T_FULL = 4096          # 2*2048 tokens
TC = T_FULL // N_CORES  # 512 tokens per core
NH = 2                 # token halves per core
TH = TC // NH          # 256 tokens per half
F = 4096
O = 4096
E = 4
R = 16
ER = E * R             # 64
M2 = ER + 2 * E        # 72 (A rows + router hi rows + router lo rows)
KB = F // 128          # 32 f-blocks
NTH = TH // 128        # 2 token chunks per half
NO = O // 512          # 8 output column chunks
NBLK = NH * 2          # 4 transpose blocks (half-major, plane-minor)
SCALE = 16.0 / np.sqrt(16.0)  # 4.0


def _build_nc():
    nc = bacc.Bacc(
        "TRN2", debug=False, target_bir_lowering=False, enable_partition_id=False
    )

    # x blocks: [h0_xh, h0_xl, h1_xh, h1_xl], each contiguous
    # [KB*TH, 128] bf16 (rows k-major, token-minor) -> one X-bar DMA each
    xhl = nc.dram_tensor("xhl", [NBLK, KB * TH, 128], BF16, kind="ExternalInput")
    # stationary packs: s[p, k*M2 + j] = S[j, k*128 + p], bf16
    s1 = nc.dram_tensor("s1", [128, KB * M2], BF16, kind="ExternalInput")
    s2 = nc.dram_tensor("s2", [128, KB * M2], BF16, kind="ExternalInput")
    # btp: [64, O]; btp[e*R+r, o] = B[e, o, r] * SCALE
    bt = nc.dram_tensor("bt", [ER, O], F32, kind="ExternalInput")
    rp = nc.dram_tensor("rp", [E, ER], F32, kind="ExternalInput")
    sm = nc.dram_tensor("sm", [2 * E, E], F32, kind="ExternalInput")  # [I4; I4]
    out = nc.dram_tensor("out", [TC, O], F32, kind="ExternalOutput")

    with tile.TileContext(nc) as tc:
        with (
            tc.tile_pool(name="xhl", bufs=NBLK) as x_pool,
            tc.tile_pool(name="consts", bufs=1) as consts,
            tc.tile_pool(name="gates", bufs=2) as gp,
            tc.tile_pool(name="outp", bufs=18) as outp,
            tc.tile_pool(name="ps_mid", bufs=2, space="PSUM") as ps_mid,
            tc.tile_pool(name="ps_lg", bufs=1, space="PSUM") as ps_lg,
            tc.tile_pool(name="ps_g", bufs=1, space="PSUM") as ps_g,
            tc.tile_pool(name="ps_d", bufs=3, space="PSUM") as ps_d,
        ):
            # ---- constants / weights (complete before the X-bar stream) ----
            ident = consts.tile([128, 128], F32)
            masks.make_identity(nc, ident[:])

            s1s = consts.tile([128, KB * M2], BF16)
            nc.scalar.dma_start(out=s1s[:], in_=s1[:])
            s2s = consts.tile([128, KB * M2], BF16)
            nc.scalar.dma_start(out=s2s[:], in_=s2[:])
            btraw = consts.tile([ER, O], F32)
            nc.sync.dma_start(out=btraw[:], in_=bt[:])
            repraw = consts.tile([E, ER], F32)
            nc.sync.dma_start(out=repraw[:], in_=rp[:])
            # summat at partitions 64:72 (must match lg8 base partition)
            sms = consts.tile([128, E], F32)
            nc.sync.dma_start(out=sms[64 : 64 + 2 * E, :], in_=sm[:])

            tc.strict_bb_all_engine_barrier()

            # f32r casts overlap the transpose window (DVE, no DMA traffic)
            bts = consts.tile([ER, O], F32R)
            nc.vector.tensor_copy(out=bts[:], in_=btraw[:])
            reps = consts.tile([E, ER], F32R)
            nc.vector.tensor_copy(out=reps[:], in_=repraw[:])

            # ---- all X-bar transposes upfront, back-to-back on sync ring ----
            xtiles = []
            last_tr = None
            for i in range(NBLK):
                xt = x_pool.tile([128, KB * TH], BF16)
                last_tr = nc.sync.dma_start(out=xt[:], in_=xhl[i], transpose=True)
                xtiles.append(xt)

            # ---- phase 1: both chains first (program order gives the
            # h1 chain priority over mm2-h0, which has copy/DMA slack) ----
            midps_list = []
            for h in range(NH):
                midps = ps_mid.tile([M2, TH], F32)  # 0:64 mid, 64:72 logits
                midps_list.append(midps)
                xck_h = xtiles[2 * h]
                xck_l = xtiles[2 * h + 1]
                for k in range(KB):
                    nc.tensor.matmul(
                        midps[:],
                        s1s[:, k * M2 : (k + 1) * M2],
                        xck_h[:, k * TH : (k + 1) * TH],
                        start=(k == 0),
                        stop=False,
                    )
                for k in range(KB):
                    nc.tensor.matmul(
                        midps[:],
                        s2s[:, k * M2 : (k + 1) * M2],
                        xck_l[:, k * TH : (k + 1) * TH],
                        start=False,
                        stop=(k == KB - 1),
                    )


            for h in range(NH):
                midps = midps_list[h]
                # ---- gates ----
                # logits = midps[64:68] + midps[68:72]; fold the sum into the
                # [e,t]->[t,e] transpose: lgt[chunk] = lg8_chunk.T @ [I4; I4]
                lg8 = gp.tile([128, TH], F32, tag="lg8")
                nc.vector.tensor_copy(out=lg8[64 : 64 + 2 * E, :], in_=midps[64:M2, :])
                lgt = ps_lg.tile([128, NTH, E], F32)
                for t in range(NTH):
                    nc.tensor.matmul(
                        lgt[:, t, :],
                        lg8[64 : 64 + 2 * E, t * 128 : (t + 1) * 128],
                        sms[64 : 64 + 2 * E, :],
                        start=True,
                        stop=True,
                    )

                # top-2 softmax, chunks batched (exp without max-shift: the
                # softmax normalization cancels it; logits are in [-7, 7])
                et = gp.tile([128, NTH, E], F32, tag="et")
                nc.scalar.activation(
                    et[:], lgt[:], mybir.ActivationFunctionType.Exp, scale=1.0
                )
                mx = gp.tile([128, NTH, 1], F32, tag="mx")
                nc.vector.reduce_max(mx[:], et[:], axis=mybir.AxisListType.X)
                m1 = gp.tile([128, NTH, E], F32, tag="m1")
                nc.vector.tensor_tensor(
                    m1[:], et[:], mx[:].to_broadcast([128, NTH, E]),
                    mybir.AluOpType.is_ge,
                )
                t1 = gp.tile([128, NTH, E], F32, tag="t1")
                nc.vector.tensor_mul(t1[:], et[:], m1[:])
                et2 = gp.tile([128, NTH, E], F32, tag="et2")
                nc.vector.tensor_sub(et2[:], et[:], t1[:])  # top-1 zeroed
                mx2 = gp.tile([128, NTH, 1], F32, tag="mx2")
                nc.vector.reduce_max(mx2[:], et2[:], axis=mybir.AxisListType.X)
                m2 = gp.tile([128, NTH, E], F32, tag="m2")
                nc.vector.tensor_tensor(
                    m2[:], et[:], mx2[:].to_broadcast([128, NTH, E]),
                    mybir.AluOpType.is_ge,
                )
                em = gp.tile([128, NTH, E], F32, tag="em")
                nc.vector.tensor_mul(em[:], et[:], m2[:])
                z = gp.tile([128, NTH, 1], F32, tag="z")
                nc.vector.reduce_sum(z[:], em[:], axis=mybir.AxisListType.X)
                rz = gp.tile([128, NTH, 1], F32, tag="rz")
                nc.vector.reciprocal(rz[:], z[:])
                g4 = gp.tile([128, NTH, E], F32, tag="g4")
                nc.vector.tensor_tensor(
                    g4[:], em[:], rz[:].to_broadcast([128, NTH, E]),
                    mybir.AluOpType.mult,
                )

                # transpose gates back -> [4e, TH], then replicate rows 4->64
                psgt = ps_g.tile([E, TH], F32, tag="psgt")
                for t in range(NTH):
                    nc.tensor.transpose(
                        psgt[:, t * 128 : (t + 1) * 128], g4[:, t, :], ident[:]
                    )
                gt = gp.tile([E, TH], F32R, tag="gt")
                nc.vector.tensor_copy(out=gt[:], in_=psgt[:])
                psrep = ps_g.tile([ER, TH], F32, tag="psrep")
                nc.tensor.matmul(psrep[:], reps[:], gt[:], start=True, stop=True)
                grep = gp.tile([ER, TH], F32, tag="grep")
                nc.scalar.copy(out=grep[:], in_=psrep[:])

                # midp = mid * gates_rep  [64, TH]
                midp = gp.tile([ER, TH], F32R, tag="midp")
                nc.vector.tensor_mul(midp[:], midps[0:ER, :], grep[:])

                # ---- phase 2: mm2 + store ----
                for t in range(NTH):
                    trow = h * TH + t * 128
                    for oc in range(NO):
                        pd = ps_d.tile([128, 512], F32)
                        nc.tensor.matmul(
                            pd[:],
                            midp[:, t * 128 : (t + 1) * 128],
                            bts[:, oc * 512 : (oc + 1) * 512],
                            start=True,
                            stop=True,
                        )
                        ob = outp.tile([128, 512], F32)
                        if oc % 2 == 0:
                            nc.vector.tensor_copy(out=ob[:], in_=pd[:])
                        else:
                            nc.scalar.copy(out=ob[:], in_=pd[:])
                        eng = nc.sync if oc % 2 == 0 else nc.scalar
                        d = eng.dma_start(
                            out=out[trow : trow + 128, oc * 512 : (oc + 1) * 512],
                            in_=ob[:],
                        )
                        # keep output DMAs strictly after the X-bar stream:
                        # a ready-early plain DMA slotted mid-stream would
                        # serialize against every remaining transpose
                        add_dep_helper(
                            d.ins, last_tr.ins, True, "defer store past xbar"
                        )
    nc.finalize()
    return nc


_NC_CACHE = None


def _get_nc():
    global _NC_CACHE
    if _NC_CACHE is None:
        _NC_CACHE = _build_nc()
    return _NC_CACHE


def _split_bf16(a):
    hi = a.astype(NP_BF16)
    lo = (a - hi.astype(np.float32)).astype(NP_BF16)
    return hi, lo


def _prep_weights(A, B, Wr):
    Acat = A.reshape(ER, F).astype(np.float32)
    Ah = Acat.astype(NP_BF16)
    wrh, wrl = _split_bf16(Wr.astype(np.float32))
    zeros = np.zeros_like(wrh)
    S1 = np.concatenate([Ah, wrh, wrl], axis=0)  # [72, F] bf16
    S2 = np.concatenate([Ah, wrh, zeros], axis=0)

    def pack(S):
        return np.ascontiguousarray(
            S.T.reshape(KB, 128, M2).transpose(1, 0, 2).reshape(128, KB * M2)
        )

    btp = np.ascontiguousarray(B.transpose(0, 2, 1).reshape(ER, O) * SCALE).astype(
        np.float32
    )
    return pack(S1), pack(S2), btp


def kernel(x, A, B, Wr, _trace=False, _trace_kwargs=None):
    x = np.asarray(x, dtype=np.float32)
    A = np.asarray(A, dtype=np.float32)
    B = np.asarray(B, dtype=np.float32)
    Wr = np.asarray(Wr, dtype=np.float32)

    orig_shape = x.shape
    flat = np.ascontiguousarray(x.reshape(-1, orig_shape[-1]))
    xh, xl = _split_bf16(flat)

    # pack [core, half, plane, k, TH, 128]: per-core blocks are
    # [h0_xh, h0_xl, h1_xh, h1_xl], each a contiguous [KB*TH, 128] X-bar src
    def pack_x(p):
        r = p.reshape(N_CORES, NH, TH, KB, 128)
        return r.transpose(0, 1, 3, 2, 4)  # [core, h, KB, TH, 128]

    xhlp = np.stack([pack_x(xh), pack_x(xl)], axis=2)  # [core, h, plane, KB, TH, 128]

    s1p, s2p, btp = _prep_weights(A, B, Wr)
    repmat = np.zeros((E, ER), dtype=np.float32)
    for e in range(E):
        repmat[e, e * R : (e + 1) * R] = 1.0
    summat = np.concatenate([np.eye(E), np.eye(E)], axis=0).astype(np.float32)

    nc = _get_nc()
    in_maps = []
    for c in range(N_CORES):
        in_maps.append(
            {
                "xhl": np.ascontiguousarray(xhlp[c]).reshape(NBLK, KB * TH, 128),
                "s1": s1p,
                "s2": s2p,
                "bt": btp,
                "rp": repmat,
                "sm": summat,
            }
        )
    kw = {}
    if _trace:
        kw = dict(trace=True, trace_cores=[0], trace_kwargs=_trace_kwargs or {})
    res = run_bass_kernel_spmd(nc, in_maps, core_ids=list(range(N_CORES)), **kw)
    outs = [res.results[c]["out"] for c in range(N_CORES)]
    full = np.concatenate(outs, axis=0).reshape(*orig_shape[:-1], O)
    kernel._last_results = res
    return full
